# revision 10
# baseline (speedup 1.0000x reference)
"""Trainium2 Bass kernel for nn_MoE_32332513804634.

MoE: 16 routed experts (top-6, softmax-then-bias routing) + dense shared
expert, T=4096 tokens, D=2048, H=1408, HS=2816, fp32.

Strategy (8 NeuronCores, SPMD):
  - Host computes the gate (cheap) and per-expert token lists.
  - Routed experts: expert token lists are carved into 128-granular pieces
    and packed into a per-core slot tuple (identical caps on every core,
    found by a small solver; ~3200 slot-tokens/core vs 3072 ideal).
  - Each slot runs SwiGLU for one expert over its gathered tokens with
    bf16 matmuls accumulating in fp32 PSUM. Weights are streamed once per
    slot (hm-outer loop, x resident in SBUF), all DMAs from host-permuted
    fully-contiguous layouts. Combine weight applied as a per-partition
    DVE scale on the PSUM->SBUF copy.
  - Shared expert is token-parallel: core c runs tokens [512c, 512c+512)
    through the full HS=2816 hidden dim (no padding).
  - Host scatters slot outputs back to token rows and adds the
    second-layer biases (cw*b2 per expert, bs2 once) in fp32.
"""

import math
import sys
import numpy as np

sys.path.insert(0, "/opt/trn_rl_repo")

import concourse.bass as bass  # noqa: E402
import concourse.tile as tile  # noqa: E402
from concourse import bacc, mybir  # noqa: E402
from concourse.bass_utils import run_bass_kernel_spmd  # noqa: E402

T = 4096
D = 2048
H = 1408
E = 16
TOP_K = 6
HS = 2816
N_CORES = 8
HM = H // 128           # 11
KO = D // 128           # 16
HMS = HS // 128         # 22
TS = T // N_CORES       # 512 shared tokens per core
F32 = mybir.dt.float32
BF16 = mybir.dt.bfloat16
MM_DT = BF16

_PROGRAM_CACHE: dict = {}


def _to_mm(a):
    import ml_dtypes
    return np.ascontiguousarray(a).astype(ml_dtypes.bfloat16)


def _host_gate(xf, gate_w, gate_b):
    """Numpy replica of the reference gate. Returns cw [T, E] dense combine
    weights and per-expert token lists (ascending)."""
    scores = xf @ gate_w.T
    m = scores.max(axis=-1, keepdims=True)
    p = np.exp(scores - m, dtype=np.float32)
    probs = p / p.sum(axis=-1, keepdims=True)
    biased = probs + gate_b
    idx = np.argpartition(biased, E - TOP_K, axis=1)[:, E - TOP_K:]
    mask = np.zeros((xf.shape[0], E), dtype=bool)
    mask[np.arange(xf.shape[0])[:, None], idx] = True
    cw = np.where(mask, probs, 0.0).astype(np.float32)
    toks = [np.flatnonzero(mask[:, e]).astype(np.int64) for e in range(E)]
    return cw, toks


def _try_pack(caps_tuple, counts):
    """Greedy-pack experts into 8 cores x caps_tuple slots (one expert per
    slot, experts may span slots/cores). Returns assignment
    [core][slot] = (expert, n) or None, or None if infeasible."""
    slots = []
    for c in range(N_CORES):
        for s, q in enumerate(caps_tuple):
            slots.append([q, c, s, None, 0])
    for e in np.argsort(counts)[::-1]:
        need = int(counts[e])
        if need == 0:
            continue
        while True:
            free = [s for s in slots if s[3] is None]
            if not free:
                return None
            free.sort(key=lambda s: -s[0])
            big = free[0]
            if need > big[0]:
                big[3] = int(e)
                big[4] = big[0]
                need -= big[0]
            else:
                r = math.ceil(need / 128) * 128
                cand = sorted((s for s in free if s[0] >= r),
                              key=lambda s: s[0])
                sl = cand[0] if cand else big
                sl[3] = int(e)
                sl[4] = need
                break
    assignment = [[None] * len(caps_tuple) for _ in range(N_CORES)]
    # hand out pieces of each expert in slot order for stable start offsets
    offs = {int(e): 0 for e in range(E)}
    for q, c, s, e, n in slots:
        if e is None or n == 0:
            continue
        assignment[c][s] = (e, offs[e], n)
        offs[e] += n
    return assignment


def _plan(counts):
    """Choose a per-core slot-cap tuple (identical across cores) minimizing
    padded tokens, then slots. Caps are 128-multiples <= 1024."""
    total = int(counts.sum())
    cmin = math.ceil(total / N_CORES / 128) * 128
    for C in range(cmin, cmin + 1024 + 1, 128):
        for S in range(2, 9):
            # descending tuples of 128-multiples <=1024 summing to C
            found = None

            def rec(rem, maxq, acc):
                nonlocal found
                if found is not None:
                    return
                k = S - len(acc)
                if k == 0:
                    if rem == 0:
                        a = _try_pack(tuple(acc), counts)
                        if a is not None:
                            found = (tuple(acc), a)
                    return
                for q in range(min(maxq, rem - 128 * (k - 1)), 127, -128):
                    if q * k < rem:
                        break
                    rec(rem - q, q, acc + [q])

            rec(C, 1024, [])
            if found:
                return found
    # fallback: uniform 512 slots
    n = math.ceil(sum(math.ceil(int(c) / 512) for c in counts) / N_CORES)
    caps = (512,) * n
    a = _try_pack(caps, counts)
    return caps, a


def _build_program(seg_caps):
    """Build the SPMD Bass program for the given per-core slot capacities.
    seg_caps: routed slot caps; shared-expert slot is appended internally."""
    nc = bacc.Bacc("TRN2", debug=False, num_devices=N_CORES)

    ins = {}
    outs = {}

    def din(name, shape, dt=MM_DT):
        ins[name] = nc.dram_tensor(name, list(shape), dt,
                                   kind="ExternalInput").ap()
        return ins[name]

    def dout(name, shape, dt=F32):
        outs[name] = nc.dram_tensor(name, list(shape), dt,
                                    kind="ExternalOutput").ap()
        return outs[name]

    for s, cap in enumerate(seg_caps):
        din(f"xg{s}", (128, KO, cap))
        din(f"w1t{s}", (HM * 128, KO, 128))
        din(f"w3t{s}", (HM * 128, KO, 128))
        din(f"w2t{s}", (128, HM, D))
        din(f"b1_{s}", (128, HM), F32)
        din(f"b3_{s}", (128, HM), F32)
        din(f"scl{s}", (128, cap // 128), F32)
        dout(f"oe{s}", (cap, D))
    din("xts", (128, KO, TS))
    din("ws1t", (HMS * 128, KO, 128))
    din("ws3t", (HMS * 128, KO, 128))
    din("ws2a0", (128, HMS, D // 2))
    din("ws2a1", (128, HMS, D // 2))
    din("bs1t", (128, HMS), F32)
    din("bs3t", (128, HMS), F32)
    dout("zs", (TS, D))

    with tile.TileContext(nc) as tc:
        with (
            tc.tile_pool(name="xpool", bufs=2) as xpool,
            tc.tile_pool(name="hpool", bufs=1) as hpool,
            tc.tile_pool(name="wcol", bufs=2) as wcol,
            tc.tile_pool(name="w2pool", bufs=2) as w2pool,
            tc.tile_pool(name="tmp", bufs=2) as tmp,
            tc.tile_pool(name="opool", bufs=3) as opool,
            tc.tile_pool(name="cpool", bufs=2) as cpool,
            tc.tile_pool(name="pp1", bufs=2, space="PSUM") as pp1,
            tc.tile_pool(name="pp2", bufs=4, space="PSUM") as pp2,
        ):
            def mlp_slot(xg_ap, w1_ap, w3_ap, w2_aps, b1_ap, b3_ap,
                         scl_ap, out_ap, cap, n_hm, scale_one):
                """One slot: out = scale * (swiglu(x) @ W2^T); biases b2/bs2
                are added on the host during the combine.
                w2_aps: list of (d0, dlen, ap) column blocks of W2^T."""
                chunks = []
                o = 0
                while o < cap:
                    c = min(512, cap - o)
                    chunks.append((o, c))
                    o += c

                b1sb = cpool.tile([128, n_hm], F32, tag="b1")
                b3sb = cpool.tile([128, n_hm], F32, tag="b3")
                nc.sync.dma_start(b1sb[:], b1_ap)
                nc.sync.dma_start(b3sb[:], b3_ap)
                if not scale_one:
                    sclsb = cpool.tile([128, cap // 128], F32, tag="scl")
                    nc.sync.dma_start(sclsb[:], scl_ap)

                # x gather on the gpsimd queue, per chunk, so the first
                # matmul starts as soon as chunk 0 + the first weight tile
                # land (the sync queue carries only w1/w3 tiles).
                xs = xpool.tile([128, KO, cap], MM_DT, tag="xg")
                for (t0, csz) in chunks:
                    nc.gpsimd.dma_start(xs[:, :, t0:t0 + csz],
                                        xg_ap[:, :, t0:t0 + csz])
                hsb = hpool.tile([128, n_hm, cap], MM_DT, tag="h")

                for hm in range(n_hm):
                    w1t_ = wcol.tile([128, KO, 128], MM_DT, tag="w1c")
                    nc.sync.dma_start(w1t_[:], w1_ap[hm * 128:(hm + 1) * 128])
                    w3t_ = wcol.tile([128, KO, 128], MM_DT, tag="w3c")
                    nc.sync.dma_start(w3t_[:], w3_ap[hm * 128:(hm + 1) * 128])
                    for (t0, csz) in chunks:
                        ps1 = pp1.tile([128, 512], F32, tag="ph1")
                        for ko in range(KO):
                            nc.tensor.matmul(ps1[:, :csz], w1t_[:, ko, :],
                                             xs[:, ko, t0:t0 + csz],
                                             start=(ko == 0),
                                             stop=(ko == KO - 1))
                        ps3 = pp1.tile([128, 512], F32, tag="ph3")
                        for ko in range(KO):
                            nc.tensor.matmul(ps3[:, :csz], w3t_[:, ko, :],
                                             xs[:, ko, t0:t0 + csz],
                                             start=(ko == 0),
                                             stop=(ko == KO - 1))
                        h1t = tmp.tile([128, 512], MM_DT, tag="h1t")
                        nc.scalar.activation(h1t[:, :csz], ps1[:, :csz],
                                             mybir.ActivationFunctionType.Silu,
                                             bias=b1sb[:, hm:hm + 1])
                        h3t = tmp.tile([128, 512], MM_DT, tag="h3t")
                        nc.scalar.activation(
                            h3t[:, :csz], ps3[:, :csz],
                            mybir.ActivationFunctionType.Identity,
                            bias=b3sb[:, hm:hm + 1])
                        nc.vector.tensor_mul(hsb[:, hm, t0:t0 + csz],
                                             h1t[:, :csz], h3t[:, :csz])

                # second matmul: out rows = tokens, streamed per W2 block
                for (d0, dlen, w2_ap) in w2_aps:
                    w2sb = w2pool.tile([128, n_hm, dlen], MM_DT, tag="w2s")
                    nc.gpsimd.dma_start(w2sb[:], w2_ap)
                    for tch in range(cap // 128):
                        tok0 = tch * 128
                        for dm in range(dlen // 512):
                            ps2 = pp2.tile([128, 512], F32, tag="po")
                            for k in range(n_hm):
                                nc.tensor.matmul(
                                    ps2[:], hsb[:, k, tok0:tok0 + 128],
                                    w2sb[:, k, dm * 512:(dm + 1) * 512],
                                    start=(k == 0), stop=(k == n_hm - 1))
                            osb = opool.tile([128, 512], F32, tag="osb")
                            if scale_one:
                                nc.vector.tensor_copy(osb[:], ps2[:])
                            else:
                                nc.vector.tensor_scalar_mul(
                                    osb[:], ps2[:], sclsb[:, tch:tch + 1])
                            nc.scalar.dma_start(
                                out_ap[tok0:tok0 + 128,
                                       d0 + dm * 512:d0 + (dm + 1) * 512],
                                osb[:])

            # shared expert first (long uniform mm1 warms the PE while the
            # routed slots' inputs stream in), then routed slots big->small
            mlp_slot(ins["xts"], ins["ws1t"], ins["ws3t"],
                     [(0, D // 2, ins["ws2a0"]), (D // 2, D // 2, ins["ws2a1"])],
                     ins["bs1t"], ins["bs3t"], None, outs["zs"], TS,
                     HMS, True)
            order = sorted(range(len(seg_caps)), key=lambda s: -seg_caps[s])
            for s in order:
                cap = seg_caps[s]
                mlp_slot(ins[f"xg{s}"], ins[f"w1t{s}"], ins[f"w3t{s}"],
                         [(0, D, ins[f"w2t{s}"])],
                         ins[f"b1_{s}"], ins[f"b3_{s}"], ins[f"scl{s}"],
                         outs[f"oe{s}"], cap, HM, False)

    nc.compile()
    return nc


def _pack_w13(w):
    """[H', D] -> [H'/128*128, KO, 128] with w1t[hm*128+p, ko, h'] =
    w[hm*128+h', ko*128+p]."""
    hm = w.shape[0] // 128
    return _to_mm(w.reshape(hm, 128, KO, 128).transpose(0, 3, 2, 1)
                  .reshape(hm * 128, KO, 128))


def _pack_w2(w):
    """[D', H'] -> [128, H'/128, D'] with w2t[p, k, d] = w[d, k*128+p]."""
    hm = w.shape[1] // 128
    return _to_mm(w.reshape(w.shape[0], hm, 128).transpose(2, 1, 0))


def _pack_x(xrows, cap):
    """[n, D] -> [128, KO, cap] with xg[p, ko, t] = x[t, ko*128+p]."""
    n = xrows.shape[0]
    out = np.zeros((128, KO, cap), np.float32)
    out[:, :, :n] = xrows.reshape(n, KO, 128).transpose(2, 1, 0)
    return _to_mm(out)


def kernel(x, gate_w, gate_b, w1, b1, w2, b2, w3, b3,
           ws1, bs1, ws2, bs2, ws3, bs3):
    x = np.asarray(x, np.float32)
    xf = np.ascontiguousarray(x.reshape(-1, D))
    gate_w = np.asarray(gate_w, np.float32)
    gate_b = np.asarray(gate_b, np.float32)
    w1 = np.asarray(w1, np.float32)
    b1 = np.asarray(b1, np.float32)
    w2 = np.asarray(w2, np.float32)
    b2 = np.asarray(b2, np.float32)
    w3 = np.asarray(w3, np.float32)
    b3 = np.asarray(b3, np.float32)
    ws1 = np.asarray(ws1, np.float32)
    bs1 = np.asarray(bs1, np.float32)
    ws2 = np.asarray(ws2, np.float32)
    bs2 = np.asarray(bs2, np.float32)
    ws3 = np.asarray(ws3, np.float32)
    bs3 = np.asarray(bs3, np.float32)

    cw, toks = _host_gate(xf, gate_w, gate_b)
    counts = np.array([len(t) for t in toks])
    seg_caps, assignment = _plan(counts)

    if seg_caps not in _PROGRAM_CACHE:
        _PROGRAM_CACHE[seg_caps] = _build_program(seg_caps)
    nc = _PROGRAM_CACHE[seg_caps]

    need = sorted({p[0] for slots in assignment for p in slots
                   if p is not None})
    w1p = {e: _pack_w13(w1[e]) for e in need}
    w3p = {e: _pack_w13(w3[e]) for e in need}
    w2p = {e: _pack_w2(w2[e]) for e in need}     # w2[e]: [D, H]
    b1p = {e: np.ascontiguousarray(b1[e].reshape(HM, 128).T) for e in need}
    b3p = {e: np.ascontiguousarray(b3[e].reshape(HM, 128).T) for e in need}

    # shared packs (same for all cores)
    ws1p = _pack_w13(ws1)
    ws3p = _pack_w13(ws3)
    ws2p = _pack_w2(ws2)                    # [128, HMS, D]
    ws2a0 = np.ascontiguousarray(ws2p[:, :, :D // 2])
    ws2a1 = np.ascontiguousarray(ws2p[:, :, D // 2:])
    bs1p = np.ascontiguousarray(bs1.reshape(HMS, 128).T)
    bs3p = np.ascontiguousarray(bs3.reshape(HMS, 128).T)

    zero_b = np.zeros((128, HM), np.float32)

    in_maps = []
    for c in range(N_CORES):
        m = {}
        for s, cap in enumerate(seg_caps):
            piece = assignment[c][s]
            scl = np.zeros(cap, np.float32)
            if piece is None:
                e = need[0]
                m[f"xg{s}"] = _pack_x(np.zeros((0, D), np.float32), cap)
                m[f"b1_{s}"] = zero_b
                m[f"b3_{s}"] = zero_b
            else:
                e, s0, n = piece
                tk = toks[e][s0:s0 + n]
                m[f"xg{s}"] = _pack_x(xf[tk], cap)
                scl[:n] = cw[tk, e]
                m[f"b1_{s}"] = b1p[e]
                m[f"b3_{s}"] = b3p[e]
            m[f"w1t{s}"] = w1p[e]
            m[f"w3t{s}"] = w3p[e]
            m[f"w2t{s}"] = w2p[e]
            m[f"scl{s}"] = np.ascontiguousarray(
                scl.reshape(cap // 128, 128).T)
        r0 = c * TS
        m["xts"] = _pack_x(xf[r0:r0 + TS], TS)
        m["ws1t"] = ws1p
        m["ws3t"] = ws3p
        m["ws2a0"] = ws2a0
        m["ws2a1"] = ws2a1
        m["bs1t"] = bs1p
        m["bs3t"] = bs3p
        in_maps.append(m)

    res = run_bass_kernel_spmd(nc, in_maps, list(range(N_CORES)))

    # host combine: scatter slot outputs + shared outputs + biases
    y = np.zeros((T, D), np.float32)
    for c in range(N_CORES):
        for s, cap in enumerate(seg_caps):
            piece = assignment[c][s]
            if piece is None:
                continue
            e, s0, n = piece
            tk = toks[e][s0:s0 + n]
            y[tk] += res.results[c][f"oe{s}"][:n]
            y[tk] += cw[tk, e][:, None] * b2[e][None, :]
        y[c * TS:(c + 1) * TS] += res.results[c]["zs"]
    y += bs2[None, :]
    return y.reshape(x.shape).astype(np.float32)


# revision 12
# speedup vs baseline: 1.0039x; 1.0039x over previous
"""Trainium2 Bass kernel for nn_MoE_32332513804634.

MoE: 16 routed experts (top-6, softmax-then-bias routing) + dense shared
expert, T=4096 tokens, D=2048, H=1408, HS=2816, fp32.

Strategy (8 NeuronCores, SPMD):
  - Host computes the gate (cheap) and per-expert token lists.
  - Routed experts: expert token lists are carved into 128-granular pieces
    and packed into a per-core slot tuple (identical caps on every core,
    found by a small solver; ~3200 slot-tokens/core vs 3072 ideal).
  - Each slot runs SwiGLU for one expert over its gathered tokens with
    bf16 matmuls accumulating in fp32 PSUM. Weights are streamed once per
    slot (hm-outer loop, x resident in SBUF), all DMAs from host-permuted
    fully-contiguous layouts. Combine weight applied as a per-partition
    DVE scale on the PSUM->SBUF copy.
  - Shared expert is token-parallel: core c runs tokens [512c, 512c+512)
    through the full HS=2816 hidden dim (no padding).
  - Host scatters slot outputs back to token rows and adds the
    second-layer biases (cw*b2 per expert, bs2 once) in fp32.
"""

import math
import sys
import numpy as np

sys.path.insert(0, "/opt/trn_rl_repo")

import concourse.bass as bass  # noqa: E402
import concourse.tile as tile  # noqa: E402
from concourse import bacc, mybir  # noqa: E402
from concourse.bass_utils import run_bass_kernel_spmd  # noqa: E402

T = 4096
D = 2048
H = 1408
E = 16
TOP_K = 6
HS = 2816
N_CORES = 8
HM = H // 128           # 11
KO = D // 128           # 16
HMS = HS // 128         # 22
TS = T // N_CORES       # 512 shared tokens per core
F32 = mybir.dt.float32
BF16 = mybir.dt.bfloat16
MM_DT = BF16

_PROGRAM_CACHE: dict = {}


def _to_mm(a):
    import ml_dtypes
    return np.ascontiguousarray(a).astype(ml_dtypes.bfloat16)


def _host_gate(xf, gate_w, gate_b):
    """Numpy replica of the reference gate. Returns cw [T, E] dense combine
    weights and per-expert token lists (ascending)."""
    scores = xf @ gate_w.T
    m = scores.max(axis=-1, keepdims=True)
    p = np.exp(scores - m, dtype=np.float32)
    probs = p / p.sum(axis=-1, keepdims=True)
    biased = probs + gate_b
    idx = np.argpartition(biased, E - TOP_K, axis=1)[:, E - TOP_K:]
    mask = np.zeros((xf.shape[0], E), dtype=bool)
    mask[np.arange(xf.shape[0])[:, None], idx] = True
    cw = np.where(mask, probs, 0.0).astype(np.float32)
    toks = [np.flatnonzero(mask[:, e]).astype(np.int64) for e in range(E)]
    return cw, toks


def _try_pack(caps_tuple, counts):
    """Greedy-pack experts into 8 cores x caps_tuple slots (one expert per
    slot, experts may span slots/cores). Returns assignment
    [core][slot] = (expert, n) or None, or None if infeasible."""
    slots = []
    for c in range(N_CORES):
        for s, q in enumerate(caps_tuple):
            slots.append([q, c, s, None, 0])
    for e in np.argsort(counts)[::-1]:
        need = int(counts[e])
        if need == 0:
            continue
        while True:
            free = [s for s in slots if s[3] is None]
            if not free:
                return None
            free.sort(key=lambda s: -s[0])
            big = free[0]
            if need > big[0]:
                big[3] = int(e)
                big[4] = big[0]
                need -= big[0]
            else:
                r = math.ceil(need / 128) * 128
                cand = sorted((s for s in free if s[0] >= r),
                              key=lambda s: s[0])
                sl = cand[0] if cand else big
                sl[3] = int(e)
                sl[4] = need
                break
    assignment = [[None] * len(caps_tuple) for _ in range(N_CORES)]
    # hand out pieces of each expert in slot order for stable start offsets
    offs = {int(e): 0 for e in range(E)}
    for q, c, s, e, n in slots:
        if e is None or n == 0:
            continue
        assignment[c][s] = (e, offs[e], n)
        offs[e] += n
    return assignment


def _plan(counts):
    """Choose a per-core slot-cap tuple (identical across cores) minimizing
    padded tokens, then slots. Caps are 128-multiples <= 1024."""
    total = int(counts.sum())
    cmin = math.ceil(total / N_CORES / 128) * 128
    for C in range(cmin, cmin + 1024 + 1, 128):
        for S in range(2, 9):
            # descending tuples of 128-multiples <=1024 summing to C
            found = None

            def rec(rem, maxq, acc):
                nonlocal found
                if found is not None:
                    return
                k = S - len(acc)
                if k == 0:
                    if rem == 0:
                        a = _try_pack(tuple(acc), counts)
                        if a is not None:
                            found = (tuple(acc), a)
                    return
                for q in range(min(maxq, rem - 128 * (k - 1)), 127, -128):
                    if q * k < rem:
                        break
                    rec(rem - q, q, acc + [q])

            rec(C, 1024, [])
            if found:
                return found
    # fallback: uniform 512 slots
    n = math.ceil(sum(math.ceil(int(c) / 512) for c in counts) / N_CORES)
    caps = (512,) * n
    a = _try_pack(caps, counts)
    return caps, a


def _build_program(seg_caps):
    """Build the SPMD Bass program for the given per-core slot capacities.
    seg_caps: routed slot caps; shared-expert slot is appended internally."""
    nc = bacc.Bacc("TRN2", debug=False, num_devices=N_CORES)

    ins = {}
    outs = {}

    def din(name, shape, dt=MM_DT):
        ins[name] = nc.dram_tensor(name, list(shape), dt,
                                   kind="ExternalInput").ap()
        return ins[name]

    def dout(name, shape, dt=F32):
        outs[name] = nc.dram_tensor(name, list(shape), dt,
                                    kind="ExternalOutput").ap()
        return outs[name]

    for s, cap in enumerate(seg_caps):
        din(f"xg{s}", (128, KO, cap))
        din(f"w1t{s}", (HM * 128, KO, 128))
        din(f"w3t{s}", (HM * 128, KO, 128))
        din(f"w2t{s}", (128, HM, D))
        din(f"b1_{s}", (128, HM), F32)
        din(f"b3_{s}", (128, HM), F32)
        din(f"scl{s}", (128, cap // 128), F32)
        dout(f"oe{s}", (cap, D))
    din("xts", (128, KO, TS))
    din("ws1t", (HMS * 128, KO, 128))
    din("ws3t", (HMS * 128, KO, 128))
    din("ws2a0", (128, HMS, D // 2))
    din("ws2a1", (128, HMS, D // 2))
    din("bs1t", (128, HMS), F32)
    din("bs3t", (128, HMS), F32)
    dout("zs", (TS, D))

    with tile.TileContext(nc) as tc:
        with (
            tc.tile_pool(name="xpool", bufs=2) as xpool,
            tc.tile_pool(name="hpool", bufs=1) as hpool,
            tc.tile_pool(name="wcol", bufs=2) as wcol,
            tc.tile_pool(name="w2pool", bufs=2) as w2pool,
            tc.tile_pool(name="tmp", bufs=2) as tmp,
            tc.tile_pool(name="opool", bufs=3) as opool,
            tc.tile_pool(name="cpool", bufs=2) as cpool,
            tc.tile_pool(name="pp1", bufs=2, space="PSUM") as pp1,
            tc.tile_pool(name="pp2", bufs=4, space="PSUM") as pp2,
        ):
            def mlp_slot(xg_ap, w1_ap, w3_ap, w2_aps, b1_ap, b3_ap,
                         scl_ap, out_ap, cap, n_hm, scale_one):
                """One slot: out = scale * (swiglu(x) @ W2^T); biases b2/bs2
                are added on the host during the combine.
                w2_aps: list of (d0, dlen, ap) column blocks of W2^T."""
                chunks = []
                o = 0
                while o < cap:
                    c = min(512, cap - o)
                    chunks.append((o, c))
                    o += c

                b1sb = cpool.tile([128, n_hm], F32, tag="b1")
                b3sb = cpool.tile([128, n_hm], F32, tag="b3")
                nc.sync.dma_start(b1sb[:], b1_ap)
                nc.sync.dma_start(b3sb[:], b3_ap)
                if not scale_one:
                    sclsb = cpool.tile([128, cap // 128], F32, tag="scl")
                    nc.sync.dma_start(sclsb[:], scl_ap)

                # x gather on the scalar (Activation) HW-DGE queue, per
                # chunk, so the first matmul starts as soon as chunk 0 +
                # the first weight tile land (the sync queue carries only
                # w1/w3 tiles; w2 and output writes ride the gpsimd queue).
                xs = xpool.tile([128, KO, cap], MM_DT, tag="xg")
                for (t0, csz) in chunks:
                    nc.scalar.dma_start(xs[:, :, t0:t0 + csz],
                                        xg_ap[:, :, t0:t0 + csz])
                hsb = hpool.tile([128, n_hm, cap], MM_DT, tag="h")

                for hm in range(n_hm):
                    w1t_ = wcol.tile([128, KO, 128], MM_DT, tag="w1c")
                    nc.sync.dma_start(w1t_[:], w1_ap[hm * 128:(hm + 1) * 128])
                    w3t_ = wcol.tile([128, KO, 128], MM_DT, tag="w3c")
                    nc.sync.dma_start(w3t_[:], w3_ap[hm * 128:(hm + 1) * 128])
                    for (t0, csz) in chunks:
                        ps1 = pp1.tile([128, 512], F32, tag="ph1")
                        for ko in range(KO):
                            nc.tensor.matmul(ps1[:, :csz], w1t_[:, ko, :],
                                             xs[:, ko, t0:t0 + csz],
                                             start=(ko == 0),
                                             stop=(ko == KO - 1))
                        ps3 = pp1.tile([128, 512], F32, tag="ph3")
                        for ko in range(KO):
                            nc.tensor.matmul(ps3[:, :csz], w3t_[:, ko, :],
                                             xs[:, ko, t0:t0 + csz],
                                             start=(ko == 0),
                                             stop=(ko == KO - 1))
                        h1t = tmp.tile([128, 512], MM_DT, tag="h1t")
                        nc.scalar.activation(h1t[:, :csz], ps1[:, :csz],
                                             mybir.ActivationFunctionType.Silu,
                                             bias=b1sb[:, hm:hm + 1])
                        h3t = tmp.tile([128, 512], MM_DT, tag="h3t")
                        nc.scalar.activation(
                            h3t[:, :csz], ps3[:, :csz],
                            mybir.ActivationFunctionType.Identity,
                            bias=b3sb[:, hm:hm + 1])
                        nc.vector.tensor_mul(hsb[:, hm, t0:t0 + csz],
                                             h1t[:, :csz], h3t[:, :csz])

                # second matmul: out rows = tokens, streamed per W2 block
                for (d0, dlen, w2_ap) in w2_aps:
                    w2sb = w2pool.tile([128, n_hm, dlen], MM_DT, tag="w2s")
                    nc.gpsimd.dma_start(w2sb[:], w2_ap)
                    for tch in range(cap // 128):
                        tok0 = tch * 128
                        for dm in range(dlen // 512):
                            ps2 = pp2.tile([128, 512], F32, tag="po")
                            for k in range(n_hm):
                                nc.tensor.matmul(
                                    ps2[:], hsb[:, k, tok0:tok0 + 128],
                                    w2sb[:, k, dm * 512:(dm + 1) * 512],
                                    start=(k == 0), stop=(k == n_hm - 1))
                            osb = opool.tile([128, 512], F32, tag="osb")
                            if scale_one:
                                nc.vector.tensor_copy(osb[:], ps2[:])
                            else:
                                nc.vector.tensor_scalar_mul(
                                    osb[:], ps2[:], sclsb[:, tch:tch + 1])
                            nc.gpsimd.dma_start(
                                out_ap[tok0:tok0 + 128,
                                       d0 + dm * 512:d0 + (dm + 1) * 512],
                                osb[:])

            # shared expert first (long uniform mm1 warms the PE while the
            # routed slots' inputs stream in), then routed slots big->small
            mlp_slot(ins["xts"], ins["ws1t"], ins["ws3t"],
                     [(0, D // 2, ins["ws2a0"]), (D // 2, D // 2, ins["ws2a1"])],
                     ins["bs1t"], ins["bs3t"], None, outs["zs"], TS,
                     HMS, True)
            order = sorted(range(len(seg_caps)), key=lambda s: -seg_caps[s])
            for s in order:
                cap = seg_caps[s]
                mlp_slot(ins[f"xg{s}"], ins[f"w1t{s}"], ins[f"w3t{s}"],
                         [(0, D, ins[f"w2t{s}"])],
                         ins[f"b1_{s}"], ins[f"b3_{s}"], ins[f"scl{s}"],
                         outs[f"oe{s}"], cap, HM, False)

    nc.compile()
    return nc


def _pack_w13(w):
    """[H', D] -> [H'/128*128, KO, 128] with w1t[hm*128+p, ko, h'] =
    w[hm*128+h', ko*128+p]."""
    hm = w.shape[0] // 128
    return _to_mm(w.reshape(hm, 128, KO, 128).transpose(0, 3, 2, 1)
                  .reshape(hm * 128, KO, 128))


def _pack_w2(w):
    """[D', H'] -> [128, H'/128, D'] with w2t[p, k, d] = w[d, k*128+p]."""
    hm = w.shape[1] // 128
    return _to_mm(w.reshape(w.shape[0], hm, 128).transpose(2, 1, 0))


def _pack_x(xrows, cap):
    """[n, D] -> [128, KO, cap] with xg[p, ko, t] = x[t, ko*128+p]."""
    n = xrows.shape[0]
    out = np.zeros((128, KO, cap), np.float32)
    out[:, :, :n] = xrows.reshape(n, KO, 128).transpose(2, 1, 0)
    return _to_mm(out)


def kernel(x, gate_w, gate_b, w1, b1, w2, b2, w3, b3,
           ws1, bs1, ws2, bs2, ws3, bs3):
    x = np.asarray(x, np.float32)
    xf = np.ascontiguousarray(x.reshape(-1, D))
    gate_w = np.asarray(gate_w, np.float32)
    gate_b = np.asarray(gate_b, np.float32)
    w1 = np.asarray(w1, np.float32)
    b1 = np.asarray(b1, np.float32)
    w2 = np.asarray(w2, np.float32)
    b2 = np.asarray(b2, np.float32)
    w3 = np.asarray(w3, np.float32)
    b3 = np.asarray(b3, np.float32)
    ws1 = np.asarray(ws1, np.float32)
    bs1 = np.asarray(bs1, np.float32)
    ws2 = np.asarray(ws2, np.float32)
    bs2 = np.asarray(bs2, np.float32)
    ws3 = np.asarray(ws3, np.float32)
    bs3 = np.asarray(bs3, np.float32)

    cw, toks = _host_gate(xf, gate_w, gate_b)
    counts = np.array([len(t) for t in toks])
    seg_caps, assignment = _plan(counts)

    if seg_caps not in _PROGRAM_CACHE:
        _PROGRAM_CACHE[seg_caps] = _build_program(seg_caps)
    nc = _PROGRAM_CACHE[seg_caps]

    need = sorted({p[0] for slots in assignment for p in slots
                   if p is not None})
    w1p = {e: _pack_w13(w1[e]) for e in need}
    w3p = {e: _pack_w13(w3[e]) for e in need}
    w2p = {e: _pack_w2(w2[e]) for e in need}     # w2[e]: [D, H]
    b1p = {e: np.ascontiguousarray(b1[e].reshape(HM, 128).T) for e in need}
    b3p = {e: np.ascontiguousarray(b3[e].reshape(HM, 128).T) for e in need}

    # shared packs (same for all cores)
    ws1p = _pack_w13(ws1)
    ws3p = _pack_w13(ws3)
    ws2p = _pack_w2(ws2)                    # [128, HMS, D]
    ws2a0 = np.ascontiguousarray(ws2p[:, :, :D // 2])
    ws2a1 = np.ascontiguousarray(ws2p[:, :, D // 2:])
    bs1p = np.ascontiguousarray(bs1.reshape(HMS, 128).T)
    bs3p = np.ascontiguousarray(bs3.reshape(HMS, 128).T)

    zero_b = np.zeros((128, HM), np.float32)

    in_maps = []
    for c in range(N_CORES):
        m = {}
        for s, cap in enumerate(seg_caps):
            piece = assignment[c][s]
            scl = np.zeros(cap, np.float32)
            if piece is None:
                e = need[0]
                m[f"xg{s}"] = _pack_x(np.zeros((0, D), np.float32), cap)
                m[f"b1_{s}"] = zero_b
                m[f"b3_{s}"] = zero_b
            else:
                e, s0, n = piece
                tk = toks[e][s0:s0 + n]
                m[f"xg{s}"] = _pack_x(xf[tk], cap)
                scl[:n] = cw[tk, e]
                m[f"b1_{s}"] = b1p[e]
                m[f"b3_{s}"] = b3p[e]
            m[f"w1t{s}"] = w1p[e]
            m[f"w3t{s}"] = w3p[e]
            m[f"w2t{s}"] = w2p[e]
            m[f"scl{s}"] = np.ascontiguousarray(
                scl.reshape(cap // 128, 128).T)
        r0 = c * TS
        m["xts"] = _pack_x(xf[r0:r0 + TS], TS)
        m["ws1t"] = ws1p
        m["ws3t"] = ws3p
        m["ws2a0"] = ws2a0
        m["ws2a1"] = ws2a1
        m["bs1t"] = bs1p
        m["bs3t"] = bs3p
        in_maps.append(m)

    res = run_bass_kernel_spmd(nc, in_maps, list(range(N_CORES)))

    # host combine: scatter slot outputs + shared outputs + biases
    y = np.zeros((T, D), np.float32)
    for c in range(N_CORES):
        for s, cap in enumerate(seg_caps):
            piece = assignment[c][s]
            if piece is None:
                continue
            e, s0, n = piece
            tk = toks[e][s0:s0 + n]
            y[tk] += res.results[c][f"oe{s}"][:n]
            y[tk] += cw[tk, e][:, None] * b2[e][None, :]
        y[c * TS:(c + 1) * TS] += res.results[c]["zs"]
    y += bs2[None, :]
    return y.reshape(x.shape).astype(np.float32)


# revision 14
# speedup vs baseline: 1.0258x; 1.0218x over previous
"""Trainium2 Bass kernel for nn_MoE_32332513804634.

MoE: 16 routed experts (top-6, softmax-then-bias routing) + dense shared
expert, T=4096 tokens, D=2048, H=1408, HS=2816, fp32.

Strategy (8 NeuronCores, SPMD):
  - Host computes the gate (cheap) and per-expert token lists.
  - Routed experts: expert token lists are carved into 128-granular pieces
    and packed into a per-core slot tuple (identical caps on every core,
    found by a small solver; ~3200 slot-tokens/core vs 3072 ideal).
  - Each slot runs SwiGLU for one expert over its gathered tokens with
    bf16 matmuls accumulating in fp32 PSUM. Weights are streamed once per
    slot (hm-outer loop, x resident in SBUF), all DMAs from host-permuted
    fully-contiguous layouts. Combine weight applied as a per-partition
    DVE scale on the PSUM->SBUF copy.
  - Shared expert is token-parallel: core c runs tokens [512c, 512c+512)
    through the full HS=2816 hidden dim (no padding).
  - Host scatters slot outputs back to token rows and adds the
    second-layer biases (cw*b2 per expert, bs2 once) in fp32.
"""

import math
import sys
import numpy as np

sys.path.insert(0, "/opt/trn_rl_repo")

import concourse.bass as bass  # noqa: E402
import concourse.tile as tile  # noqa: E402
from concourse import bacc, mybir  # noqa: E402
from concourse.bass_utils import run_bass_kernel_spmd  # noqa: E402

T = 4096
D = 2048
H = 1408
E = 16
TOP_K = 6
HS = 2816
N_CORES = 8
HM = H // 128           # 11
KO = D // 128           # 16
HMS = HS // 128         # 22
TS = T // N_CORES       # 512 shared tokens per core
F32 = mybir.dt.float32
BF16 = mybir.dt.bfloat16
MM_DT = BF16

_PROGRAM_CACHE: dict = {}


def _to_mm(a):
    import ml_dtypes
    return np.ascontiguousarray(a).astype(ml_dtypes.bfloat16)


def _host_gate(xf, gate_w, gate_b):
    """Numpy replica of the reference gate. Returns cw [T, E] dense combine
    weights and per-expert token lists (ascending)."""
    scores = xf @ gate_w.T
    m = scores.max(axis=-1, keepdims=True)
    p = np.exp(scores - m, dtype=np.float32)
    probs = p / p.sum(axis=-1, keepdims=True)
    biased = probs + gate_b
    idx = np.argpartition(biased, E - TOP_K, axis=1)[:, E - TOP_K:]
    mask = np.zeros((xf.shape[0], E), dtype=bool)
    mask[np.arange(xf.shape[0])[:, None], idx] = True
    cw = np.where(mask, probs, 0.0).astype(np.float32)
    toks = [np.flatnonzero(mask[:, e]).astype(np.int64) for e in range(E)]
    return cw, toks


def _try_pack(caps_tuple, counts):
    """Greedy-pack experts into 8 cores x caps_tuple slots (one expert per
    slot, experts may span slots/cores). Returns assignment
    [core][slot] = (expert, n) or None, or None if infeasible."""
    slots = []
    for c in range(N_CORES):
        for s, q in enumerate(caps_tuple):
            slots.append([q, c, s, None, 0])
    for e in np.argsort(counts)[::-1]:
        need = int(counts[e])
        if need == 0:
            continue
        while True:
            free = [s for s in slots if s[3] is None]
            if not free:
                return None
            free.sort(key=lambda s: -s[0])
            big = free[0]
            if need > big[0]:
                big[3] = int(e)
                big[4] = big[0]
                need -= big[0]
            else:
                r = math.ceil(need / 128) * 128
                cand = sorted((s for s in free if s[0] >= r),
                              key=lambda s: s[0])
                sl = cand[0] if cand else big
                sl[3] = int(e)
                sl[4] = need
                break
    assignment = [[None] * len(caps_tuple) for _ in range(N_CORES)]
    # hand out pieces of each expert in slot order for stable start offsets
    offs = {int(e): 0 for e in range(E)}
    for q, c, s, e, n in slots:
        if e is None or n == 0:
            continue
        assignment[c][s] = (e, offs[e], n)
        offs[e] += n
    return assignment


def _plan(counts):
    """Choose a per-core slot-cap tuple (identical across cores) minimizing
    padded tokens, then slots. Caps are 128-multiples <= 1024."""
    total = int(counts.sum())
    cmin = math.ceil(total / N_CORES / 128) * 128
    for C in range(cmin, cmin + 1024 + 1, 128):
        for S in range(2, 9):
            # descending tuples of 128-multiples <=1024 summing to C
            found = None

            def rec(rem, maxq, acc):
                nonlocal found
                if found is not None:
                    return
                k = S - len(acc)
                if k == 0:
                    if rem == 0:
                        a = _try_pack(tuple(acc), counts)
                        if a is not None:
                            found = (tuple(acc), a)
                    return
                for q in range(min(maxq, rem - 128 * (k - 1)), 127, -128):
                    if q * k < rem:
                        break
                    rec(rem - q, q, acc + [q])

            rec(C, 1024, [])
            if found:
                return found
    # fallback: uniform 512 slots
    n = math.ceil(sum(math.ceil(int(c) / 512) for c in counts) / N_CORES)
    caps = (512,) * n
    a = _try_pack(caps, counts)
    return caps, a


def _build_program(seg_caps):
    """Build the SPMD Bass program for the given per-core slot capacities.
    seg_caps: routed slot caps; shared-expert slot is appended internally."""
    nc = bacc.Bacc("TRN2", debug=False, num_devices=N_CORES)

    ins = {}
    outs = {}

    def din(name, shape, dt=MM_DT):
        ins[name] = nc.dram_tensor(name, list(shape), dt,
                                   kind="ExternalInput").ap()
        return ins[name]

    def dout(name, shape, dt=F32):
        outs[name] = nc.dram_tensor(name, list(shape), dt,
                                    kind="ExternalOutput").ap()
        return outs[name]

    for s, cap in enumerate(seg_caps):
        din(f"xg{s}", (128, KO, cap))
        din(f"w1t{s}", (HM * 128, KO, 128))
        din(f"w3t{s}", (HM * 128, KO, 128))
        din(f"w2t{s}", (128, HM, D))
        din(f"b1_{s}", (128, HM), F32)
        din(f"b3_{s}", (128, HM), F32)
        din(f"scl{s}", (128, cap // 128), F32)
        dout(f"oe{s}", (cap, D))
    din("xts", (128, KO, TS))
    din("ws1t", (HMS * 128, KO, 128))
    din("ws3t", (HMS * 128, KO, 128))
    din("ws2a0", (128, HMS, D // 2))
    din("ws2a1", (128, HMS, D // 2))
    din("bs1t", (128, HMS), F32)
    din("bs3t", (128, HMS), F32)
    dout("zs", (TS, D))

    with tile.TileContext(nc) as tc:
        with (
            tc.tile_pool(name="xpool", bufs=2) as xpool,
            tc.tile_pool(name="hpool", bufs=1) as hpool,
            tc.tile_pool(name="wcol", bufs=2) as wcol,
            tc.tile_pool(name="w2pool", bufs=4) as w2pool,
            tc.tile_pool(name="tmp", bufs=2) as tmp,
            tc.tile_pool(name="opool", bufs=3) as opool,
            tc.tile_pool(name="cpool", bufs=2) as cpool,
            tc.tile_pool(name="pp1", bufs=2, space="PSUM") as pp1,
            tc.tile_pool(name="pp2", bufs=4, space="PSUM") as pp2,
        ):
            def mlp_slot(xg_ap, w1_ap, w3_ap, w2_aps, b1_ap, b3_ap,
                         scl_ap, out_ap, cap, n_hm, scale_one):
                """One slot: out = scale * (swiglu(x) @ W2^T); biases b2/bs2
                are added on the host during the combine.
                w2_aps: list of (d0, dlen, ap) column blocks of W2^T."""
                chunks = []
                o = 0
                while o < cap:
                    c = min(512, cap - o)
                    chunks.append((o, c))
                    o += c

                b1sb = cpool.tile([128, n_hm], F32, tag="b1")
                b3sb = cpool.tile([128, n_hm], F32, tag="b3")
                nc.sync.dma_start(b1sb[:], b1_ap)
                nc.sync.dma_start(b3sb[:], b3_ap)
                if not scale_one:
                    sclsb = cpool.tile([128, cap // 128], F32, tag="scl")
                    nc.sync.dma_start(sclsb[:], scl_ap)

                # x gather on the scalar (Activation) HW-DGE queue, per
                # chunk, so the first matmul starts as soon as chunk 0 +
                # the first weight tile land (the sync queue carries only
                # w1/w3 tiles; w2 and output writes ride the gpsimd queue).
                xs = xpool.tile([128, KO, cap], MM_DT, tag="xg")
                for (t0, csz) in chunks:
                    nc.scalar.dma_start(xs[:, :, t0:t0 + csz],
                                        xg_ap[:, :, t0:t0 + csz])
                hsb = hpool.tile([128, n_hm, cap], MM_DT, tag="h")

                for hm in range(n_hm):
                    w1t_ = wcol.tile([128, KO, 128], MM_DT, tag="w1c")
                    nc.sync.dma_start(w1t_[:], w1_ap[hm * 128:(hm + 1) * 128])
                    w3t_ = wcol.tile([128, KO, 128], MM_DT, tag="w3c")
                    nc.sync.dma_start(w3t_[:], w3_ap[hm * 128:(hm + 1) * 128])
                    for (t0, csz) in chunks:
                        ps1 = pp1.tile([128, 512], F32, tag="ph1")
                        for ko in range(KO):
                            nc.tensor.matmul(ps1[:, :csz], w1t_[:, ko, :],
                                             xs[:, ko, t0:t0 + csz],
                                             start=(ko == 0),
                                             stop=(ko == KO - 1))
                        ps3 = pp1.tile([128, 512], F32, tag="ph3")
                        for ko in range(KO):
                            nc.tensor.matmul(ps3[:, :csz], w3t_[:, ko, :],
                                             xs[:, ko, t0:t0 + csz],
                                             start=(ko == 0),
                                             stop=(ko == KO - 1))
                        h1t = tmp.tile([128, 512], MM_DT, tag="h1t")
                        nc.scalar.activation(h1t[:, :csz], ps1[:, :csz],
                                             mybir.ActivationFunctionType.Silu,
                                             bias=b1sb[:, hm:hm + 1])
                        h3t = tmp.tile([128, 512], MM_DT, tag="h3t")
                        nc.scalar.activation(
                            h3t[:, :csz], ps3[:, :csz],
                            mybir.ActivationFunctionType.Identity,
                            bias=b3sb[:, hm:hm + 1])
                        nc.vector.tensor_mul(hsb[:, hm, t0:t0 + csz],
                                             h1t[:, :csz], h3t[:, :csz])

                # second matmul: out rows = tokens, streamed per W2 block
                for (d0, dlen, w2_ap) in w2_aps:
                    w2sb = w2pool.tile([128, n_hm, dlen], MM_DT, tag="w2s")
                    nc.gpsimd.dma_start(w2sb[:], w2_ap)
                    for tch in range(cap // 128):
                        tok0 = tch * 128
                        for dm in range(dlen // 512):
                            ps2 = pp2.tile([128, 512], F32, tag="po")
                            for k in range(n_hm):
                                nc.tensor.matmul(
                                    ps2[:], hsb[:, k, tok0:tok0 + 128],
                                    w2sb[:, k, dm * 512:(dm + 1) * 512],
                                    start=(k == 0), stop=(k == n_hm - 1))
                            osb = opool.tile([128, 512], F32, tag="osb")
                            if scale_one:
                                nc.vector.tensor_copy(osb[:], ps2[:])
                            else:
                                nc.vector.tensor_scalar_mul(
                                    osb[:], ps2[:], sclsb[:, tch:tch + 1])
                            nc.gpsimd.dma_start(
                                out_ap[tok0:tok0 + 128,
                                       d0 + dm * 512:d0 + (dm + 1) * 512],
                                osb[:])

            # W2 is streamed in 1024-wide d-blocks (uniform 22.5 KB tiles,
            # 4 pool bufs = ~2 slots of prefetch depth on the gpsimd queue).
            def w2blocks(ap, n_hm):
                step = 1024 if n_hm == HM else 512
                return [(d0, step, ap[:, :, d0:d0 + step])
                        for d0 in range(0, ap.shape[-1], step)]

            def routed(s):
                cap = seg_caps[s]
                mlp_slot(ins[f"xg{s}"], ins[f"w1t{s}"], ins[f"w3t{s}"],
                         w2blocks(ins[f"w2t{s}"], HM),
                         ins[f"b1_{s}"], ins[f"b3_{s}"], ins[f"scl{s}"],
                         outs[f"oe{s}"], cap, HM, False)

            def shared():
                blocks = (w2blocks(ins["ws2a0"], HMS)
                          + [(d0 + D // 2, dl, ap) for (d0, dl, ap)
                             in w2blocks(ins["ws2a1"], HMS)])
                mlp_slot(ins["xts"], ins["ws1t"], ins["ws3t"], blocks,
                         ins["bs1t"], ins["bs3t"], None, outs["zs"], TS,
                         HMS, True)

            # order: a mid-size slot first (small critical-path x DMA, but
            # a long enough mm1 to cover the following streams), big slots
            # and the shared expert in the middle, smallest slots last
            # (their W2 prefetches run far ahead on the gpsimd queue).
            order = sorted(range(len(seg_caps)), key=lambda s: -seg_caps[s])
            mid = [s for s in order if seg_caps[s] == 512]
            first = mid[0] if mid else order[0]
            rest = [s for s in order if s != first]
            big = [s for s in rest if seg_caps[s] > 512]
            small = [s for s in rest if seg_caps[s] <= 512]
            routed(first)
            for s in big:
                routed(s)
            shared()
            for s in small:
                routed(s)

    nc.compile()
    return nc


def _pack_w13(w):
    """[H', D] -> [H'/128*128, KO, 128] with w1t[hm*128+p, ko, h'] =
    w[hm*128+h', ko*128+p]."""
    hm = w.shape[0] // 128
    return _to_mm(w.reshape(hm, 128, KO, 128).transpose(0, 3, 2, 1)
                  .reshape(hm * 128, KO, 128))


def _pack_w2(w):
    """[D', H'] -> [128, H'/128, D'] with w2t[p, k, d] = w[d, k*128+p]."""
    hm = w.shape[1] // 128
    return _to_mm(w.reshape(w.shape[0], hm, 128).transpose(2, 1, 0))


def _pack_x(xrows, cap):
    """[n, D] -> [128, KO, cap] with xg[p, ko, t] = x[t, ko*128+p]."""
    n = xrows.shape[0]
    out = np.zeros((128, KO, cap), np.float32)
    out[:, :, :n] = xrows.reshape(n, KO, 128).transpose(2, 1, 0)
    return _to_mm(out)


def kernel(x, gate_w, gate_b, w1, b1, w2, b2, w3, b3,
           ws1, bs1, ws2, bs2, ws3, bs3):
    x = np.asarray(x, np.float32)
    xf = np.ascontiguousarray(x.reshape(-1, D))
    gate_w = np.asarray(gate_w, np.float32)
    gate_b = np.asarray(gate_b, np.float32)
    w1 = np.asarray(w1, np.float32)
    b1 = np.asarray(b1, np.float32)
    w2 = np.asarray(w2, np.float32)
    b2 = np.asarray(b2, np.float32)
    w3 = np.asarray(w3, np.float32)
    b3 = np.asarray(b3, np.float32)
    ws1 = np.asarray(ws1, np.float32)
    bs1 = np.asarray(bs1, np.float32)
    ws2 = np.asarray(ws2, np.float32)
    bs2 = np.asarray(bs2, np.float32)
    ws3 = np.asarray(ws3, np.float32)
    bs3 = np.asarray(bs3, np.float32)

    cw, toks = _host_gate(xf, gate_w, gate_b)
    counts = np.array([len(t) for t in toks])
    seg_caps, assignment = _plan(counts)

    if seg_caps not in _PROGRAM_CACHE:
        _PROGRAM_CACHE[seg_caps] = _build_program(seg_caps)
    nc = _PROGRAM_CACHE[seg_caps]

    need = sorted({p[0] for slots in assignment for p in slots
                   if p is not None})
    w1p = {e: _pack_w13(w1[e]) for e in need}
    w3p = {e: _pack_w13(w3[e]) for e in need}
    w2p = {e: _pack_w2(w2[e]) for e in need}     # w2[e]: [D, H]
    b1p = {e: np.ascontiguousarray(b1[e].reshape(HM, 128).T) for e in need}
    b3p = {e: np.ascontiguousarray(b3[e].reshape(HM, 128).T) for e in need}

    # shared packs (same for all cores)
    ws1p = _pack_w13(ws1)
    ws3p = _pack_w13(ws3)
    ws2p = _pack_w2(ws2)                    # [128, HMS, D]
    ws2a0 = np.ascontiguousarray(ws2p[:, :, :D // 2])
    ws2a1 = np.ascontiguousarray(ws2p[:, :, D // 2:])
    bs1p = np.ascontiguousarray(bs1.reshape(HMS, 128).T)
    bs3p = np.ascontiguousarray(bs3.reshape(HMS, 128).T)

    zero_b = np.zeros((128, HM), np.float32)

    in_maps = []
    for c in range(N_CORES):
        m = {}
        for s, cap in enumerate(seg_caps):
            piece = assignment[c][s]
            scl = np.zeros(cap, np.float32)
            if piece is None:
                e = need[0]
                m[f"xg{s}"] = _pack_x(np.zeros((0, D), np.float32), cap)
                m[f"b1_{s}"] = zero_b
                m[f"b3_{s}"] = zero_b
            else:
                e, s0, n = piece
                tk = toks[e][s0:s0 + n]
                m[f"xg{s}"] = _pack_x(xf[tk], cap)
                scl[:n] = cw[tk, e]
                m[f"b1_{s}"] = b1p[e]
                m[f"b3_{s}"] = b3p[e]
            m[f"w1t{s}"] = w1p[e]
            m[f"w3t{s}"] = w3p[e]
            m[f"w2t{s}"] = w2p[e]
            m[f"scl{s}"] = np.ascontiguousarray(
                scl.reshape(cap // 128, 128).T)
        r0 = c * TS
        m["xts"] = _pack_x(xf[r0:r0 + TS], TS)
        m["ws1t"] = ws1p
        m["ws3t"] = ws3p
        m["ws2a0"] = ws2a0
        m["ws2a1"] = ws2a1
        m["bs1t"] = bs1p
        m["bs3t"] = bs3p
        in_maps.append(m)

    res = run_bass_kernel_spmd(nc, in_maps, list(range(N_CORES)))

    # host combine: scatter slot outputs + shared outputs + biases
    y = np.zeros((T, D), np.float32)
    for c in range(N_CORES):
        for s, cap in enumerate(seg_caps):
            piece = assignment[c][s]
            if piece is None:
                continue
            e, s0, n = piece
            tk = toks[e][s0:s0 + n]
            y[tk] += res.results[c][f"oe{s}"][:n]
            y[tk] += cw[tk, e][:, None] * b2[e][None, :]
        y[c * TS:(c + 1) * TS] += res.results[c]["zs"]
    y += bs2[None, :]
    return y.reshape(x.shape).astype(np.float32)


# revision 15
# speedup vs baseline: 1.0368x; 1.0108x over previous
"""Trainium2 Bass kernel for nn_MoE_32332513804634.

MoE: 16 routed experts (top-6, softmax-then-bias routing) + dense shared
expert, T=4096 tokens, D=2048, H=1408, HS=2816, fp32.

Strategy (8 NeuronCores, SPMD):
  - Host computes the gate (cheap) and per-expert token lists.
  - Routed experts: expert token lists are carved into 128-granular pieces
    and packed into a per-core slot tuple (identical caps on every core,
    found by a small solver; ~3200 slot-tokens/core vs 3072 ideal).
  - Each slot runs SwiGLU for one expert over its gathered tokens with
    bf16 matmuls accumulating in fp32 PSUM. Weights are streamed once per
    slot (hm-outer loop, x resident in SBUF), all DMAs from host-permuted
    fully-contiguous layouts. Combine weight applied as a per-partition
    DVE scale on the PSUM->SBUF copy.
  - Shared expert is token-parallel: core c runs tokens [512c, 512c+512)
    through the full HS=2816 hidden dim (no padding).
  - Host scatters slot outputs back to token rows and adds the
    second-layer biases (cw*b2 per expert, bs2 once) in fp32.
"""

import math
import sys
import numpy as np

sys.path.insert(0, "/opt/trn_rl_repo")

import concourse.bass as bass  # noqa: E402
import concourse.tile as tile  # noqa: E402
from concourse import bacc, mybir  # noqa: E402
from concourse.bass_utils import run_bass_kernel_spmd  # noqa: E402

T = 4096
D = 2048
H = 1408
E = 16
TOP_K = 6
HS = 2816
N_CORES = 8
HM = H // 128           # 11
KO = D // 128           # 16
HMS = HS // 128         # 22
TS = T // N_CORES       # 512 shared tokens per core
F32 = mybir.dt.float32
BF16 = mybir.dt.bfloat16
MM_DT = BF16

_PROGRAM_CACHE: dict = {}


def _to_mm(a):
    import ml_dtypes
    return np.ascontiguousarray(a).astype(ml_dtypes.bfloat16)


def _host_gate(xf, gate_w, gate_b):
    """Numpy replica of the reference gate. Returns cw [T, E] dense combine
    weights and per-expert token lists (ascending)."""
    scores = xf @ gate_w.T
    m = scores.max(axis=-1, keepdims=True)
    p = np.exp(scores - m, dtype=np.float32)
    probs = p / p.sum(axis=-1, keepdims=True)
    biased = probs + gate_b
    idx = np.argpartition(biased, E - TOP_K, axis=1)[:, E - TOP_K:]
    mask = np.zeros((xf.shape[0], E), dtype=bool)
    mask[np.arange(xf.shape[0])[:, None], idx] = True
    cw = np.where(mask, probs, 0.0).astype(np.float32)
    toks = [np.flatnonzero(mask[:, e]).astype(np.int64) for e in range(E)]
    return cw, toks


def _try_pack(caps_tuple, counts):
    """Greedy-pack experts into 8 cores x caps_tuple slots (one expert per
    slot, experts may span slots/cores). Returns assignment
    [core][slot] = (expert, n) or None, or None if infeasible."""
    slots = []
    for c in range(N_CORES):
        for s, q in enumerate(caps_tuple):
            slots.append([q, c, s, None, 0])
    for e in np.argsort(counts)[::-1]:
        need = int(counts[e])
        if need == 0:
            continue
        while True:
            free = [s for s in slots if s[3] is None]
            if not free:
                return None
            free.sort(key=lambda s: -s[0])
            big = free[0]
            if need > big[0]:
                big[3] = int(e)
                big[4] = big[0]
                need -= big[0]
            else:
                r = math.ceil(need / 128) * 128
                cand = sorted((s for s in free if s[0] >= r),
                              key=lambda s: s[0])
                sl = cand[0] if cand else big
                sl[3] = int(e)
                sl[4] = need
                break
    assignment = [[None] * len(caps_tuple) for _ in range(N_CORES)]
    # hand out pieces of each expert in slot order for stable start offsets
    offs = {int(e): 0 for e in range(E)}
    for q, c, s, e, n in slots:
        if e is None or n == 0:
            continue
        assignment[c][s] = (e, offs[e], n)
        offs[e] += n
    return assignment


def _plan(counts):
    """Choose a per-core slot-cap tuple (identical across cores) minimizing
    padded tokens, then slots. Caps are 128-multiples <= 1024."""
    total = int(counts.sum())
    cmin = math.ceil(total / N_CORES / 128) * 128
    for C in range(cmin, cmin + 1024 + 1, 128):
        for S in range(2, 9):
            # descending tuples of 128-multiples <=1024 summing to C
            found = None

            def rec(rem, maxq, acc):
                nonlocal found
                if found is not None:
                    return
                k = S - len(acc)
                if k == 0:
                    if rem == 0:
                        a = _try_pack(tuple(acc), counts)
                        if a is not None:
                            found = (tuple(acc), a)
                    return
                for q in range(min(maxq, rem - 128 * (k - 1)), 127, -128):
                    if q * k < rem:
                        break
                    rec(rem - q, q, acc + [q])

            rec(C, 1024, [])
            if found:
                return found
    # fallback: uniform 512 slots
    n = math.ceil(sum(math.ceil(int(c) / 512) for c in counts) / N_CORES)
    caps = (512,) * n
    a = _try_pack(caps, counts)
    return caps, a


def _build_program(seg_caps):
    """Build the SPMD Bass program for the given per-core slot capacities.
    seg_caps: routed slot caps; shared-expert slot is appended internally."""
    nc = bacc.Bacc("TRN2", debug=False, num_devices=N_CORES)

    ins = {}
    outs = {}

    def din(name, shape, dt=MM_DT):
        ins[name] = nc.dram_tensor(name, list(shape), dt,
                                   kind="ExternalInput").ap()
        return ins[name]

    def dout(name, shape, dt=F32):
        outs[name] = nc.dram_tensor(name, list(shape), dt,
                                    kind="ExternalOutput").ap()
        return outs[name]

    for s, cap in enumerate(seg_caps):
        din(f"xg{s}", (128, KO, cap))
        din(f"w1t{s}", (HM * 128, KO, 128))
        din(f"w3t{s}", (HM * 128, KO, 128))
        din(f"w2t{s}", (128, HM, D))
        din(f"b1_{s}", (128, HM), F32)
        din(f"b3_{s}", (128, HM), F32)
        din(f"scl{s}", (128, cap // 128), F32)
        dout(f"oe{s}", (cap, D))
    din("xts", (128, KO, TS))
    din("ws1t", (HMS * 128, KO, 128))
    din("ws3t", (HMS * 128, KO, 128))
    din("ws2a0", (128, HMS, D // 2))
    din("ws2a1", (128, HMS, D // 2))
    din("bs1t", (128, HMS), F32)
    din("bs3t", (128, HMS), F32)
    dout("zs", (TS, D))

    with tile.TileContext(nc) as tc:
        with (
            tc.tile_pool(name="xpool", bufs=2) as xpool,
            tc.tile_pool(name="hpool", bufs=1) as hpool,
            tc.tile_pool(name="wcol", bufs=2) as wcol,
            tc.tile_pool(name="w2pool", bufs=4) as w2pool,
            tc.tile_pool(name="tmp", bufs=2) as tmp,
            tc.tile_pool(name="opool", bufs=3) as opool,
            tc.tile_pool(name="cpool", bufs=2) as cpool,
            tc.tile_pool(name="pp1", bufs=2, space="PSUM") as pp1,
            tc.tile_pool(name="pp2", bufs=4, space="PSUM") as pp2,
        ):
            def mlp_slot(xg_ap, w1_ap, w3_ap, w2_aps, b1_ap, b3_ap,
                         scl_ap, out_ap, cap, n_hm, scale_one):
                """One slot: out = scale * (swiglu(x) @ W2^T); biases b2/bs2
                are added on the host during the combine.
                w2_aps: list of (d0, dlen, ap) column blocks of W2^T."""
                chunks = []
                o = 0
                while o < cap:
                    c = min(512, cap - o)
                    chunks.append((o, c))
                    o += c

                # x gather per chunk ahead of this slot's weight tiles on
                # the sync queue (scalar queue must stay free for the
                # ACTIVATEs; w2 and output writes ride the gpsimd queue).
                xs = xpool.tile([128, KO, cap], MM_DT, tag="xg")
                for (t0, csz) in chunks:
                    nc.sync.dma_start(xs[:, :, t0:t0 + csz],
                                      xg_ap[:, :, t0:t0 + csz])

                b1sb = cpool.tile([128, n_hm], F32, tag="b1")
                b3sb = cpool.tile([128, n_hm], F32, tag="b3")
                nc.sync.dma_start(b1sb[:], b1_ap)
                nc.sync.dma_start(b3sb[:], b3_ap)
                if not scale_one:
                    sclsb = cpool.tile([128, cap // 128], F32, tag="scl")
                    nc.sync.dma_start(sclsb[:], scl_ap)
                hsb = hpool.tile([128, n_hm, cap], MM_DT, tag="h")

                for hm in range(n_hm):
                    w1t_ = wcol.tile([128, KO, 128], MM_DT, tag="w1c")
                    nc.sync.dma_start(w1t_[:], w1_ap[hm * 128:(hm + 1) * 128])
                    w3t_ = wcol.tile([128, KO, 128], MM_DT, tag="w3c")
                    nc.sync.dma_start(w3t_[:], w3_ap[hm * 128:(hm + 1) * 128])
                    for (t0, csz) in chunks:
                        ps1 = pp1.tile([128, 512], F32, tag="ph1")
                        for ko in range(KO):
                            nc.tensor.matmul(ps1[:, :csz], w1t_[:, ko, :],
                                             xs[:, ko, t0:t0 + csz],
                                             start=(ko == 0),
                                             stop=(ko == KO - 1))
                        ps3 = pp1.tile([128, 512], F32, tag="ph3")
                        for ko in range(KO):
                            nc.tensor.matmul(ps3[:, :csz], w3t_[:, ko, :],
                                             xs[:, ko, t0:t0 + csz],
                                             start=(ko == 0),
                                             stop=(ko == KO - 1))
                        h1t = tmp.tile([128, 512], MM_DT, tag="h1t")
                        nc.scalar.activation(h1t[:, :csz], ps1[:, :csz],
                                             mybir.ActivationFunctionType.Silu,
                                             bias=b1sb[:, hm:hm + 1])
                        h3t = tmp.tile([128, 512], MM_DT, tag="h3t")
                        nc.scalar.activation(
                            h3t[:, :csz], ps3[:, :csz],
                            mybir.ActivationFunctionType.Identity,
                            bias=b3sb[:, hm:hm + 1])
                        nc.vector.tensor_mul(hsb[:, hm, t0:t0 + csz],
                                             h1t[:, :csz], h3t[:, :csz])

                # second matmul: out rows = tokens, streamed per W2 block
                for (d0, dlen, w2_ap) in w2_aps:
                    w2sb = w2pool.tile([128, n_hm, dlen], MM_DT, tag="w2s")
                    nc.gpsimd.dma_start(w2sb[:], w2_ap)
                    for tch in range(cap // 128):
                        tok0 = tch * 128
                        for dm in range(dlen // 512):
                            ps2 = pp2.tile([128, 512], F32, tag="po")
                            for k in range(n_hm):
                                nc.tensor.matmul(
                                    ps2[:], hsb[:, k, tok0:tok0 + 128],
                                    w2sb[:, k, dm * 512:(dm + 1) * 512],
                                    start=(k == 0), stop=(k == n_hm - 1))
                            osb = opool.tile([128, 512], F32, tag="osb")
                            if scale_one:
                                nc.vector.tensor_copy(osb[:], ps2[:])
                            else:
                                nc.vector.tensor_scalar_mul(
                                    osb[:], ps2[:], sclsb[:, tch:tch + 1])
                            nc.gpsimd.dma_start(
                                out_ap[tok0:tok0 + 128,
                                       d0 + dm * 512:d0 + (dm + 1) * 512],
                                osb[:])

            # W2 is streamed in 1024-wide d-blocks (uniform 22.5 KB tiles,
            # 4 pool bufs = ~2 slots of prefetch depth on the gpsimd queue).
            def w2blocks(ap, n_hm):
                step = 1024 if n_hm == HM else 512
                return [(d0, step, ap[:, :, d0:d0 + step])
                        for d0 in range(0, ap.shape[-1], step)]

            def routed(s):
                cap = seg_caps[s]
                mlp_slot(ins[f"xg{s}"], ins[f"w1t{s}"], ins[f"w3t{s}"],
                         w2blocks(ins[f"w2t{s}"], HM),
                         ins[f"b1_{s}"], ins[f"b3_{s}"], ins[f"scl{s}"],
                         outs[f"oe{s}"], cap, HM, False)

            def shared():
                blocks = (w2blocks(ins["ws2a0"], HMS)
                          + [(d0 + D // 2, dl, ap) for (d0, dl, ap)
                             in w2blocks(ins["ws2a1"], HMS)])
                mlp_slot(ins["xts"], ins["ws1t"], ins["ws3t"], blocks,
                         ins["bs1t"], ins["bs3t"], None, outs["zs"], TS,
                         HMS, True)

            # order: a mid-size slot first (small critical-path x DMA, but
            # a long enough mm1 to cover the following streams), big slots
            # and the shared expert in the middle, smallest slots last
            # (their W2 prefetches run far ahead on the gpsimd queue).
            order = sorted(range(len(seg_caps)), key=lambda s: -seg_caps[s])
            mid = [s for s in order if seg_caps[s] == 512]
            first = mid[0] if mid else order[0]
            rest = [s for s in order if s != first]
            big = [s for s in rest if seg_caps[s] > 512]
            small = [s for s in rest if seg_caps[s] <= 512]
            routed(first)
            for s in big:
                routed(s)
            shared()
            for s in small:
                routed(s)

    nc.compile()
    return nc


def _pack_w13(w):
    """[H', D] -> [H'/128*128, KO, 128] with w1t[hm*128+p, ko, h'] =
    w[hm*128+h', ko*128+p]."""
    hm = w.shape[0] // 128
    return _to_mm(w.reshape(hm, 128, KO, 128).transpose(0, 3, 2, 1)
                  .reshape(hm * 128, KO, 128))


def _pack_w2(w):
    """[D', H'] -> [128, H'/128, D'] with w2t[p, k, d] = w[d, k*128+p]."""
    hm = w.shape[1] // 128
    return _to_mm(w.reshape(w.shape[0], hm, 128).transpose(2, 1, 0))


def _pack_x(xrows, cap):
    """[n, D] -> [128, KO, cap] with xg[p, ko, t] = x[t, ko*128+p]."""
    n = xrows.shape[0]
    out = np.zeros((128, KO, cap), np.float32)
    out[:, :, :n] = xrows.reshape(n, KO, 128).transpose(2, 1, 0)
    return _to_mm(out)


def kernel(x, gate_w, gate_b, w1, b1, w2, b2, w3, b3,
           ws1, bs1, ws2, bs2, ws3, bs3):
    x = np.asarray(x, np.float32)
    xf = np.ascontiguousarray(x.reshape(-1, D))
    gate_w = np.asarray(gate_w, np.float32)
    gate_b = np.asarray(gate_b, np.float32)
    w1 = np.asarray(w1, np.float32)
    b1 = np.asarray(b1, np.float32)
    w2 = np.asarray(w2, np.float32)
    b2 = np.asarray(b2, np.float32)
    w3 = np.asarray(w3, np.float32)
    b3 = np.asarray(b3, np.float32)
    ws1 = np.asarray(ws1, np.float32)
    bs1 = np.asarray(bs1, np.float32)
    ws2 = np.asarray(ws2, np.float32)
    bs2 = np.asarray(bs2, np.float32)
    ws3 = np.asarray(ws3, np.float32)
    bs3 = np.asarray(bs3, np.float32)

    cw, toks = _host_gate(xf, gate_w, gate_b)
    counts = np.array([len(t) for t in toks])
    seg_caps, assignment = _plan(counts)

    if seg_caps not in _PROGRAM_CACHE:
        _PROGRAM_CACHE[seg_caps] = _build_program(seg_caps)
    nc = _PROGRAM_CACHE[seg_caps]

    need = sorted({p[0] for slots in assignment for p in slots
                   if p is not None})
    w1p = {e: _pack_w13(w1[e]) for e in need}
    w3p = {e: _pack_w13(w3[e]) for e in need}
    w2p = {e: _pack_w2(w2[e]) for e in need}     # w2[e]: [D, H]
    b1p = {e: np.ascontiguousarray(b1[e].reshape(HM, 128).T) for e in need}
    b3p = {e: np.ascontiguousarray(b3[e].reshape(HM, 128).T) for e in need}

    # shared packs (same for all cores)
    ws1p = _pack_w13(ws1)
    ws3p = _pack_w13(ws3)
    ws2p = _pack_w2(ws2)                    # [128, HMS, D]
    ws2a0 = np.ascontiguousarray(ws2p[:, :, :D // 2])
    ws2a1 = np.ascontiguousarray(ws2p[:, :, D // 2:])
    bs1p = np.ascontiguousarray(bs1.reshape(HMS, 128).T)
    bs3p = np.ascontiguousarray(bs3.reshape(HMS, 128).T)

    zero_b = np.zeros((128, HM), np.float32)

    in_maps = []
    for c in range(N_CORES):
        m = {}
        for s, cap in enumerate(seg_caps):
            piece = assignment[c][s]
            scl = np.zeros(cap, np.float32)
            if piece is None:
                e = need[0]
                m[f"xg{s}"] = _pack_x(np.zeros((0, D), np.float32), cap)
                m[f"b1_{s}"] = zero_b
                m[f"b3_{s}"] = zero_b
            else:
                e, s0, n = piece
                tk = toks[e][s0:s0 + n]
                m[f"xg{s}"] = _pack_x(xf[tk], cap)
                scl[:n] = cw[tk, e]
                m[f"b1_{s}"] = b1p[e]
                m[f"b3_{s}"] = b3p[e]
            m[f"w1t{s}"] = w1p[e]
            m[f"w3t{s}"] = w3p[e]
            m[f"w2t{s}"] = w2p[e]
            m[f"scl{s}"] = np.ascontiguousarray(
                scl.reshape(cap // 128, 128).T)
        r0 = c * TS
        m["xts"] = _pack_x(xf[r0:r0 + TS], TS)
        m["ws1t"] = ws1p
        m["ws3t"] = ws3p
        m["ws2a0"] = ws2a0
        m["ws2a1"] = ws2a1
        m["bs1t"] = bs1p
        m["bs3t"] = bs3p
        in_maps.append(m)

    res = run_bass_kernel_spmd(nc, in_maps, list(range(N_CORES)))

    # host combine: scatter slot outputs + shared outputs + biases
    y = np.zeros((T, D), np.float32)
    for c in range(N_CORES):
        for s, cap in enumerate(seg_caps):
            piece = assignment[c][s]
            if piece is None:
                continue
            e, s0, n = piece
            tk = toks[e][s0:s0 + n]
            y[tk] += res.results[c][f"oe{s}"][:n]
            y[tk] += cw[tk, e][:, None] * b2[e][None, :]
        y[c * TS:(c + 1) * TS] += res.results[c]["zs"]
    y += bs2[None, :]
    return y.reshape(x.shape).astype(np.float32)
